# revision 11
# baseline (speedup 1.0000x reference)
"""Trainium2 Bass kernel for nn_Baka_84791244358183.

Math (reference):
    coeff  = weight[:, :, 0]            # [O, I]
    powers = weight[:, :, 1:]           # [O, I, J]   (J == I == 256)
    out[b, o] = sum_f coeff[o, f] * exp( sum_j log(x[b, j]) * powers[o, f, j] )

Shapes: x [B=1024, I=256], weight [O=512, I=256, 257], out [B, O].

Strategy: tensor-parallel over O across 8 cores (64 outputs each).
Per core, for each output feature o:
  stage 1 (PE, fp8 DoubleRow): mm[f, b] = sum_j powers[o,f,j] * logx[j, b]
  stage 2 (ACT):               pf = exp(mm)          (bf16, PSUM -> SBUF)
  stage 3 (PE):                out[o, b] = sum_f coeff[o,f] * pf[f, b]
Stage 3 is an M=1 matmul; four consecutive o's land in one PSUM bank at
partitions {0,32,64,96} via tile_position col-groups, are copied out as a
full 128-partition tile, and leave via a partition-strided DMA.
Host packs/reorders the weights (fp8 DoubleRow layout), device computes
log(x) once, all compute + reductions run on device.
"""

import numpy as np
import ml_dtypes

B = 1024
I_FEAT = 256  # output-feature dim of the inner product ("i" in the einsum)
J = 256       # contraction dim (log-x features)
O = 512
NCORES = 8
OPC = O // NCORES  # 64 outputs per core

_CACHE: dict = {}


def _build_bass():
    import concourse.bass as bass
    import concourse.tile as tile
    from concourse import bacc, mybir

    f32 = mybir.dt.float32
    f8 = mybir.dt.float8e4
    bf16 = mybir.dt.bfloat16
    AF = mybir.ActivationFunctionType
    DR = mybir.MatmulPerfMode.DoubleRow

    nc = bacc.Bacc()

    xt_d = nc.declare_dram_parameter("xt", [128, 2, B], f32, isOutput=False)
    pw_d = nc.declare_dram_parameter("pw", [128, OPC, 2, I_FEAT], f8, isOutput=False)
    cf_d = nc.declare_dram_parameter("cf", [128, OPC, 2], bf16, isOutput=False)
    out_d = nc.declare_dram_parameter("outT", [OPC, B], f32, isOutput=True)

    with tile.TileContext(nc) as tc:
        with (
            tc.tile_pool(name="const", bufs=1) as const_pool,
            tc.tile_pool(name="pf", bufs=3) as pf_pool,
            tc.tile_pool(name="stage", bufs=4) as stage_pool,
            tc.tile_pool(name="ps1", bufs=2, space="PSUM") as ps1_pool,
            tc.tile_pool(name="ps2", bufs=1, space="PSUM") as ps2_pool,
        ):
            xt_sb = const_pool.tile([128, 2, B], f32)
            logx = const_pool.tile([128, 2, B], f8)
            pw_sb = const_pool.tile([128, OPC, 2, I_FEAT], f8)
            cf_sb = const_pool.tile([128, OPC, 2], bf16)

            nc.sync.dma_start(xt_sb[:], xt_d[:])
            nc.sync.dma_start(cf_sb[:], cf_d[:])
            # weights in 8 chunks so compute can start early
            for g in range(8):
                sl = slice(g * (OPC // 8), (g + 1) * (OPC // 8))
                nc.sync.dma_start(pw_sb[:, sl], pw_d[:, sl])

            # logx[kj, kt, b] = ln(x[b, kt*128+kj]), stored fp8 for DoubleRow
            nc.scalar.activation(logx[:], xt_sb[:], AF.Ln)

            # Persistent stage-3 accumulator banks (2-deep by quad parity x
            # 2 b-chunks). Zero once via a zero-matmul so the full-bank DVE
            # copy below never reads uninitialized PSUM; later quads only
            # rewrite partitions {0,32,64,96}.
            z16 = const_pool.tile([128, 512], bf16)
            nc.gpsimd.memset(z16[:], 0.0)
            ps2q_t = {}
            for par in range(2):
                for bc in range(2):
                    t = ps2_pool.tile(
                        [128, 512], f32, name=f"ps2q_{par}_{bc}", tag=f"q{par}{bc}"
                    )
                    nc.tensor.matmul(
                        t[:], lhsT=z16[:, 0:128], rhs=z16[:], start=True, stop=True
                    )
                    ps2q_t[(par, bc)] = t

            def stage1(o):
                pf = pf_pool.tile([128, 2, B], bf16)
                for ft in range(2):
                    ps1 = ps1_pool.tile([128, B], f32)
                    for bc in range(2):
                        nc.tensor.matmul(
                            ps1[:, bc * 512:(bc + 1) * 512],
                            lhsT=pw_sb[:, o, :, ft * 128:(ft + 1) * 128],
                            rhs=logx[:, :, bc * 512:(bc + 1) * 512],
                            start=True,
                            stop=True,
                            perf_mode=DR,
                        )
                    nc.scalar.activation(pf[:, ft, :], ps1[:], AF.Exp)
                return pf

            def stage3(o, pf):
                q, r = divmod(o, 4)
                par = q % 2
                for bc in range(2):
                    for ft in range(2):
                        nc.tensor.matmul(
                            ps2q_t[(par, bc)][32 * r:32 * r + 1, :],
                            lhsT=cf_sb[:, o, ft:ft + 1],
                            rhs=pf[:, ft, bc * 512:(bc + 1) * 512],
                            start=(ft == 0),
                            stop=(ft == 1),
                            tile_position=(0, 32 * r),
                        )
                if r == 3:
                    for bc in range(2):
                        st = stage_pool.tile([128, 512], f32)
                        nc.vector.tensor_copy(st[:], ps2q_t[(par, bc)][:])
                        nc.sync.dma_start(
                            out_d[4 * q:4 * (q + 1), bc * 512:(bc + 1) * 512],
                            st[0:128:32, :],
                        )

            prev = None
            for o in range(OPC):
                pf = stage1(o)
                if prev is not None:
                    stage3(*prev)
                prev = (o, pf)
            stage3(*prev)

    nc.compile()
    return nc


def _get_nc():
    if "nc" not in _CACHE:
        _CACHE["nc"] = _build_bass()
    return _CACHE["nc"]


def make_in_maps(x: np.ndarray, weight: np.ndarray):
    x = np.asarray(x, dtype=np.float32)
    weight = np.asarray(weight, dtype=np.float32)
    # xt[kj, kt, b] = x[b, kt*128+kj]
    xt = np.ascontiguousarray(x.T.reshape(2, 128, B).transpose(1, 0, 2))
    in_maps = []
    for c in range(NCORES):
        osl = slice(c * OPC, (c + 1) * OPC)
        p = weight[osl, :, 1:]  # [OPC, f, j]
        pw = np.ascontiguousarray(
            p.reshape(OPC, I_FEAT, 2, 128).transpose(3, 0, 2, 1)
        ).astype(ml_dtypes.float8_e4m3)  # [kj, o, kt, f]
        cfm = weight[osl, :, 0]  # [OPC, f]
        cf = np.ascontiguousarray(
            cfm.reshape(OPC, 2, 128).transpose(2, 0, 1)
        ).astype(ml_dtypes.bfloat16)  # [fp, o, ft]
        in_maps.append({"xt": xt, "pw": pw, "cf": cf})
    return in_maps


def kernel(x: np.ndarray, weight: np.ndarray) -> np.ndarray:
    from concourse.bass_utils import run_bass_kernel_spmd

    nc = _get_nc()
    in_maps = make_in_maps(x, weight)
    res = run_bass_kernel_spmd(nc, in_maps, list(range(NCORES))).results
    outT = np.concatenate([res[c]["outT"] for c in range(NCORES)], axis=0)
    return np.ascontiguousarray(outT.T).astype(np.float32)  # [B, O]


if __name__ == "__main__":
    # quick CoreSim check on core 0 against a numpy oracle
    from concourse.bass_interp import CoreSim

    rng = np.random.default_rng(0)
    x = (rng.random((B, I_FEAT), dtype=np.float32) + 0.1)
    weight = rng.standard_normal((O, I_FEAT, J + 1), dtype=np.float32) * 0.05
    # non-degenerate powers so the sim check exercises real numerics
    weight[:, :, 1:] = rng.random((O, I_FEAT, J), dtype=np.float32) * 0.02

    nc = _get_nc()
    in_maps = make_in_maps(x, weight)

    sim = CoreSim(nc)
    for k, v in in_maps[0].items():
        sim.tensor(k)[:] = v
    sim.simulate()
    got = np.array(sim.tensor("outT"))  # [OPC, B]

    logx = np.log(x)
    coeff = weight[:OPC, :, 0]
    powers = weight[:OPC, :, 1:]
    mm = np.einsum("bj,ofj->obf", logx, powers)
    pf = np.exp(mm)
    want = np.einsum("obf,of->ob", pf, coeff)  # [OPC, B]

    err = np.abs(got - want)
    rel = np.linalg.norm(got - want) / np.linalg.norm(want)
    print("want abs max:", np.abs(want).max())
    print("max abs err:", err.max())
    print("fro rel err:", rel)


# revision 20
# speedup vs baseline: 1.0576x; 1.0576x over previous
"""Trainium2 Bass kernel for nn_Baka_84791244358183.

Math (reference):
    coeff  = weight[:, :, 0]            # [O, I]
    powers = weight[:, :, 1:]           # [O, I, J]   (J == I == 256)
    out[b, o] = sum_f coeff[o, f] * exp( sum_j log(x[b, j]) * powers[o, f, j] )

Shapes: x [B=1024, I=256], weight [O=512, I=256, 257], out [B, O].

Strategy: tensor-parallel over O across 8 cores (64 outputs each).
Per core, for each output feature o:
  stage 1 (PE, fp8 DoubleRow): mm[f, b] = sum_j powers[o,f,j] * logx[j, b]
  stage 2 (ACT):               pf = exp(mm)          (bf16, PSUM -> SBUF)
  stage 3 (PE):                out[o, b] = sum_f coeff[o,f] * pf[f, b]
Stage 3 is an M=1 matmul; four consecutive o's land in one PSUM bank at
partitions {0,32,64,96} via tile_position col-groups, are copied out as a
full 128-partition tile, and leave via a partition-strided DMA.
Host packs/reorders the weights (fp8 DoubleRow layout), device computes
log(x) once, all compute + reductions run on device.
"""

import numpy as np
import ml_dtypes

B = 1024
I_FEAT = 256  # output-feature dim of the inner product ("i" in the einsum)
J = 256       # contraction dim (log-x features)
O = 512
NCORES = 8
OPC = O // NCORES  # 64 outputs per core

_CACHE: dict = {}


def _build_bass():
    import concourse.bass as bass
    import concourse.tile as tile
    from concourse import bacc, mybir

    f32 = mybir.dt.float32
    f8 = mybir.dt.float8e4
    bf16 = mybir.dt.bfloat16
    AF = mybir.ActivationFunctionType
    DR = mybir.MatmulPerfMode.DoubleRow

    nc = bacc.Bacc()

    xt_d = nc.declare_dram_parameter("xt", [128, 2, B], f32, isOutput=False)
    pw_d = nc.declare_dram_parameter("pw", [128, OPC, 2, I_FEAT], f8, isOutput=False)
    cf_d = nc.declare_dram_parameter("cf", [128, OPC, 2, 128], f8, isOutput=False)
    out_d = nc.declare_dram_parameter("outT", [OPC, B], f32, isOutput=True)

    with tile.TileContext(nc) as tc:
        with (
            tc.tile_pool(name="const", bufs=1) as const_pool,
            tc.tile_pool(name="pf", bufs=3) as pf_pool,
            tc.tile_pool(name="stage", bufs=4) as stage_pool,
            tc.tile_pool(name="ps1", bufs=2, space="PSUM") as ps1_pool,
            tc.tile_pool(name="ps2", bufs=1, space="PSUM") as ps2_pool,
        ):
            xt_sb = const_pool.tile([128, 2, B], f32)
            logx = const_pool.tile([128, 2, B], f8)
            pw_sb = const_pool.tile([128, OPC, 2, I_FEAT], f8)
            cf_sb = const_pool.tile([128, OPC, 2, 128], f8)

            nc.sync.dma_start(xt_sb[:], xt_d[:])
            nc.sync.dma_start(cf_sb[:], cf_d[:])
            # weights in 8 chunks so compute can start early
            for g in range(8):
                sl = slice(g * (OPC // 8), (g + 1) * (OPC // 8))
                nc.sync.dma_start(pw_sb[:, sl], pw_d[:, sl])

            # Warm the ACT Ln table while the input DMA is in flight so the
            # real ln doesn't pay the ~1.3us table load serially.
            warm = const_pool.tile([128, 1], f32)
            nc.gpsimd.memset(warm[:], 1.0)
            nc.scalar.activation(warm[:], warm[:], AF.Ln)

            # logx[kj, kt, b] = ln(x[b, kt*128+kj]), stored fp8 for DoubleRow
            nc.scalar.activation(logx[:], xt_sb[:], AF.Ln)

            # Persistent stage-3 accumulator banks (2-deep by quad parity x
            # 2 b-chunks). Each quad's r==0 matmul start=True overwrites the
            # whole bank, so no explicit zero-init is needed.
            ps2q_t = {}
            for par in range(2):
                for bc in range(2):
                    t = ps2_pool.tile(
                        [128, 512], f32, name=f"ps2q_{par}_{bc}", tag=f"q{par}{bc}"
                    )
                    ps2q_t[(par, bc)] = t

            def stage1(o):
                pf = pf_pool.tile([128, 2, B], f8)
                for ft in range(2):
                    ps1 = ps1_pool.tile([128, B], f32)
                    for bc in range(2):
                        nc.tensor.matmul(
                            ps1[:, bc * 512:(bc + 1) * 512],
                            lhsT=pw_sb[:, o, :, ft * 128:(ft + 1) * 128],
                            rhs=logx[:, :, bc * 512:(bc + 1) * 512],
                            start=True,
                            stop=True,
                            perf_mode=DR,
                        )
                    nc.scalar.activation(pf[:, ft, :], ps1[:], AF.Exp)
                return pf

            def stage3(o, pf):
                q, r = divmod(o, 4)
                par = q % 2
                # Full-array DR matmul: the coeff pair sits in lhsT column
                # 32*r, so o's output lands on PSUM partition 32*r; all other
                # lhsT columns are zero and accumulate 0 onto the other rows.
                for bc in range(2):
                    nc.tensor.matmul(
                        ps2q_t[(par, bc)][:, :],
                        lhsT=cf_sb[:, o, :, :],
                        rhs=pf[:, :, bc * 512:(bc + 1) * 512],
                        start=(r == 0),
                        stop=(r == 3),
                        perf_mode=DR,
                    )
                if r == 3:
                    for bc in range(2):
                        st = stage_pool.tile([128, 512], f32)
                        nc.vector.tensor_copy(st[:], ps2q_t[(par, bc)][:])
                        nc.sync.dma_start(
                            out_d[4 * q:4 * (q + 1), bc * 512:(bc + 1) * 512],
                            st[0:128:32, :],
                        )

            prev = None
            for o in range(OPC):
                pf = stage1(o)
                if prev is not None:
                    stage3(*prev)
                prev = (o, pf)
            stage3(*prev)

    nc.compile()
    return nc


def _get_nc():
    if "nc" not in _CACHE:
        _CACHE["nc"] = _build_bass()
    return _CACHE["nc"]


def make_in_maps(x: np.ndarray, weight: np.ndarray):
    x = np.asarray(x, dtype=np.float32)
    weight = np.asarray(weight, dtype=np.float32)
    # xt[kj, kt, b] = x[b, kt*128+kj]
    xt = np.ascontiguousarray(x.T.reshape(2, 128, B).transpose(1, 0, 2))
    in_maps = []
    for c in range(NCORES):
        osl = slice(c * OPC, (c + 1) * OPC)
        p = weight[osl, :, 1:]  # [OPC, f, j]
        pw = np.ascontiguousarray(
            p.reshape(OPC, I_FEAT, 2, 128).transpose(3, 0, 2, 1)
        ).astype(ml_dtypes.float8_e4m3)  # [kj, o, kt, f]
        cfm = weight[osl, :, 0]  # [OPC, f]
        # [fp, o, ft, 128]: coeff pair in column 32*(o%4), zeros elsewhere;
        # the stage-3 full-array DR matmul then drops o's output on PSUM
        # partition 32*(o%4) with zero contribution to the other partitions.
        cf = np.zeros((128, OPC, 2, 128), dtype=ml_dtypes.float8_e4m3)
        cfq = cfm.reshape(OPC, 2, 128).transpose(2, 0, 1).astype(
            ml_dtypes.float8_e4m3
        )
        for o in range(OPC):
            cf[:, o, :, 32 * (o % 4)] = cfq[:, o, :]
        in_maps.append({"xt": xt, "pw": pw, "cf": cf})
    return in_maps


def kernel(x: np.ndarray, weight: np.ndarray) -> np.ndarray:
    from concourse.bass_utils import run_bass_kernel_spmd

    nc = _get_nc()
    in_maps = make_in_maps(x, weight)
    res = run_bass_kernel_spmd(nc, in_maps, list(range(NCORES))).results
    outT = np.concatenate([res[c]["outT"] for c in range(NCORES)], axis=0)
    return np.ascontiguousarray(outT.T).astype(np.float32)  # [B, O]


if __name__ == "__main__":
    # quick CoreSim check on core 0 against a numpy oracle
    from concourse.bass_interp import CoreSim

    rng = np.random.default_rng(0)
    x = (rng.random((B, I_FEAT), dtype=np.float32) + 0.1)
    weight = rng.standard_normal((O, I_FEAT, J + 1), dtype=np.float32) * 0.05
    # non-degenerate powers so the sim check exercises real numerics
    weight[:, :, 1:] = rng.random((O, I_FEAT, J), dtype=np.float32) * 0.02

    nc = _get_nc()
    in_maps = make_in_maps(x, weight)

    sim = CoreSim(nc)
    for k, v in in_maps[0].items():
        sim.tensor(k)[:] = v
    sim.simulate()
    got = np.array(sim.tensor("outT"))  # [OPC, B]

    logx = np.log(x)
    coeff = weight[:OPC, :, 0]
    powers = weight[:OPC, :, 1:]
    mm = np.einsum("bj,ofj->obf", logx, powers)
    pf = np.exp(mm)
    want = np.einsum("obf,of->ob", pf, coeff)  # [OPC, B]

    err = np.abs(got - want)
    rel = np.linalg.norm(got - want) / np.linalg.norm(want)
    print("want abs max:", np.abs(want).max())
    print("max abs err:", err.max())
    print("fro rel err:", rel)
